# revision 5
# baseline (speedup 1.0000x reference)
"""GraphTransformerEncoder on 8 trn2 NeuronCores (Bass/Tile).

Sharding: nodes are split contiguously across 8 cores (6250 each, padded to
6272 = 49*128). Within a core, nodes are relabeled by degree (descending) so
each 128-node group has a tight max in-degree. Edges live on their
destination-node owner core. Per layer, each core projects K/V for its own
nodes, an AllGather builds the full interleaved KV table, and the edge stage
gathers source rows by index (indirect DMA), does segment softmax + aggregation
with nodes on partitions and degree-slots streamed in chunks, then the gated
skip connection, GELU and LayerNorm — all fully local.

The edge embedding term (edge_attr @ We) is folded in analytically:
  alpha = (q.(k_src+e))/sqrt(C) = q.k_src/sqrt(C) + (q.We/sqrt(C)) * ea
  out_num = sum ex*(v_src+e) = sum ex*v_src + (sum ex*ea) * We_row
so K/V rows are gathered raw and no per-edge embedding is materialized.
"""
import sys
sys.path.insert(0, "/opt/trn_rl_repo")

import numpy as np

import concourse.bass as bass
import concourse.bacc as bacc
import concourse.tile as tile
from concourse import mybir
from concourse import bass_utils
from concourse.masks import make_identity

N = 50000
E = 800000
IN = 256
HID = 128
H = 4
C = 32
L = 3
P = 128
NCORES = 8
NLOC = N // NCORES            # 6250
NT = (NLOC + P - 1) // P      # 49 node tiles per core
NPAD = NT * P                 # 6272
NFULL = NCORES * NPAD         # 50176
JW = 8                        # max degree-slots per chunk
RSC = 1.0 / np.sqrt(C)

F32 = mybir.dt.float32
I32 = mybir.dt.int32


def _preprocess(x, edge_index, edge_attr):
    """Host-side sharding. Returns per-core arrays + metadata."""
    src = edge_index[0].astype(np.int64)
    dst = edge_index[1].astype(np.int64)
    ea = edge_attr[:, 0].astype(np.float32)

    deg = np.bincount(dst, minlength=N)
    perm = np.empty((NCORES, NLOC), np.int64)   # new local idx -> orig local idx
    invp = np.empty(N, np.int64)                # orig global -> new local idx
    for c in range(NCORES):
        d = deg[c * NLOC:(c + 1) * NLOC]
        p = np.argsort(-d, kind="stable")
        perm[c] = p
        invp[c * NLOC + p] = np.arange(NLOC)
    new_gid = (np.arange(N) // NLOC) * NPAD + invp

    owner = dst // NLOC
    dst_nl = invp[dst]
    src_ng = new_gid[src]

    # per-core per-group degree maxima (padded nodes have degree 0)
    cnts = np.zeros((NCORES, NPAD), np.int64)
    for c in range(NCORES):
        m = owner == c
        cnts[c] = np.bincount(dst_nl[m], minlength=NPAD)
    Dg = cnts.reshape(NCORES, NT, P).max(axis=2).max(axis=0)  # [NT]
    offs = np.zeros(NT + 1, np.int64)
    offs[1:] = np.cumsum(Dg)
    SUMD = int(offs[-1])

    gidx = np.zeros((NCORES, P, SUMD), np.int32)
    eav = np.zeros((NCORES, P, SUMD), np.float32)
    msk = np.zeros((NCORES, P, SUMD), np.float32)
    for c in range(NCORES):
        m = owner == c
        dl = dst_nl[m]
        sg = src_ng[m]
        ev = ea[m]
        order = np.argsort(dl, kind="stable")
        dls = dl[order]
        sgs = sg[order]
        eas = ev[order]
        counts = cnts[c]
        starts = np.zeros(NPAD, np.int64)
        starts[1:] = np.cumsum(counts)[:-1]
        jidx = np.arange(len(dls)) - starts[dls]
        grp = dls // P
        lane = dls % P
        col = offs[grp] + jidx
        gidx[c, lane, col] = sgs.astype(np.int32)
        eav[c, lane, col] = eas
        msk[c, lane, col] = 1.0

    xT = np.zeros((NCORES, IN, NPAD), np.float32)
    for c in range(NCORES):
        xT[c, :, :NLOC] = x[c * NLOC + perm[c]].T

    return xT, gidx, eav, msk, Dg, offs, SUMD, perm


def _chunks(d):
    """Split degree d into chunks of width <= JW."""
    out = []
    while d > 0:
        w = min(JW, d)
        out.append(w)
        d -= w
    return out


_CACHE = {}


def _build(Dg, offs, SUMD):
    key = (tuple(Dg), SUMD)
    if key in _CACHE:
        return _CACHE[key]

    nc = bacc.Bacc("TRN2", target_bir_lowering=False, debug=False,
                   num_devices=NCORES)

    # ---- kernel I/O ----
    xT_d = nc.dram_tensor("xT", [IN, NPAD], F32, kind="ExternalInput").ap()
    gidx_d = nc.dram_tensor("gidx", [P, SUMD], I32, kind="ExternalInput").ap()
    eav_d = nc.dram_tensor("eav", [P, SUMD], F32, kind="ExternalInput").ap()
    msk_d = nc.dram_tensor("msk", [P, SUMD], F32, kind="ExternalInput").ap()
    Wi_d = nc.dram_tensor("Wi", [IN, HID], F32, kind="ExternalInput").ap()
    bi_d = nc.dram_tensor("bi", [1, HID], F32, kind="ExternalInput").ap()
    Wq_d = nc.dram_tensor("Wq", [L, HID, HID], F32, kind="ExternalInput").ap()
    Wk_d = nc.dram_tensor("Wk", [L, HID, HID], F32, kind="ExternalInput").ap()
    Wv_d = nc.dram_tensor("Wv", [L, HID, HID], F32, kind="ExternalInput").ap()
    Ws_d = nc.dram_tensor("Ws", [L, HID, HID], F32, kind="ExternalInput").ap()
    bq_d = nc.dram_tensor("bq", [L, 1, HID], F32, kind="ExternalInput").ap()
    bk_d = nc.dram_tensor("bk", [L, 1, HID], F32, kind="ExternalInput").ap()
    bv_d = nc.dram_tensor("bv", [L, 1, HID], F32, kind="ExternalInput").ap()
    bs_d = nc.dram_tensor("bs", [L, 1, HID], F32, kind="ExternalInput").ap()
    WeR_d = nc.dram_tensor("WeR", [L, P, HID], F32, kind="ExternalInput").ap()
    wAR_d = nc.dram_tensor("wAR", [L, P, HID], F32, kind="ExternalInput").ap()
    wBR_d = nc.dram_tensor("wBR", [L, P, HID], F32, kind="ExternalInput").ap()
    lgR_d = nc.dram_tensor("lgR", [L, P, HID], F32, kind="ExternalInput").ap()
    lbR_d = nc.dram_tensor("lbR", [L, P, HID], F32, kind="ExternalInput").ap()
    out_d = nc.dram_tensor("out_h", [NPAD, HID], F32, kind="ExternalOutput").ap()

    kv_in = nc.dram_tensor("kv_in", [NPAD, 2 * HID], F32).ap()
    kv_full = nc.dram_tensor("kv_full", [NFULL, 2 * HID], F32,
                             addr_space="Shared").ap()

    cc_sem = nc.alloc_semaphore(name="cc_sem")

    # ---- persistent SBUF ----
    h_sb = nc.alloc_sbuf_tensor("h_sb", [P, NPAD], F32).ap()
    q_sb = nc.alloc_sbuf_tensor("q_sb", [P, NPAD], F32).ap()
    s_sb = nc.alloc_sbuf_tensor("s_sb", [P, NPAD], F32).ap()
    Wi_sb = nc.alloc_sbuf_tensor("Wi_sb", [P, 2 * HID], F32).ap()
    W_sb = nc.alloc_sbuf_tensor("W_sb", [P, 4 * L * HID], F32).ap()  # q,k,v,s per layer
    bias_sb = nc.alloc_sbuf_tensor("bias_sb", [1, (4 * L + 1) * HID], F32).ap()
    rep_sb = nc.alloc_sbuf_tensor("rep_sb", [P, 5 * L * HID], F32).ap()  # WeR,wAR,wBR,lgR,lbR
    ones_sb = nc.alloc_sbuf_tensor("ones_sb", [1, HID], F32).ap()
    eps_sb = nc.alloc_sbuf_tensor("eps_sb", [P, 1], F32).ap()
    ident = nc.alloc_sbuf_tensor("ident", [P, P], F32).ap()

    def Wslice(kind, l):  # kind: 0=q 1=k 2=v 3=s
        c0 = (l * 4 + kind) * HID
        return W_sb[:, c0:c0 + HID]

    def bslice(kind, l):
        c0 = (l * 4 + kind) * HID
        return bias_sb[:, c0:c0 + HID]

    bi_sl = bias_sb[:, 4 * L * HID:(4 * L + 1) * HID]

    def repslice(kind, l):  # 0=WeR 1=wAR 2=wBR 3=lgR 4=lbR
        c0 = (l * 5 + kind) * HID
        return rep_sb[:, c0:c0 + HID]

    # ================= stage 0: consts + input projection =================
    with tile.TileContext(nc) as tc:
        make_identity(nc, ident)
        nc.vector.memset(ones_sb, 1.0)
        nc.vector.memset(eps_sb, 1e-5)
        nc.sync.dma_start(out=Wi_sb[:, 0:HID], in_=Wi_d[0:P, :])
        nc.sync.dma_start(out=Wi_sb[:, HID:2 * HID], in_=Wi_d[P:2 * P, :])
        nc.sync.dma_start(out=bi_sl, in_=bi_d[:])
        for l in range(L):
            nc.sync.dma_start(out=Wslice(0, l), in_=Wq_d[l])
            nc.sync.dma_start(out=Wslice(1, l), in_=Wk_d[l])
            nc.sync.dma_start(out=Wslice(2, l), in_=Wv_d[l])
            nc.sync.dma_start(out=Wslice(3, l), in_=Ws_d[l])
            nc.sync.dma_start(out=bslice(0, l), in_=bq_d[l])
            nc.sync.dma_start(out=bslice(1, l), in_=bk_d[l])
            nc.sync.dma_start(out=bslice(2, l), in_=bv_d[l])
            nc.sync.dma_start(out=bslice(3, l), in_=bs_d[l])
            nc.sync.dma_start(out=repslice(0, l), in_=WeR_d[l])
            nc.sync.dma_start(out=repslice(1, l), in_=wAR_d[l])
            nc.sync.dma_start(out=repslice(2, l), in_=wBR_d[l])
            nc.sync.dma_start(out=repslice(3, l), in_=lgR_d[l])
            nc.sync.dma_start(out=repslice(4, l), in_=lbR_d[l])
        with tc.tile_pool(name="s0", bufs=3) as pool, \
             tc.tile_pool(name="s0p", bufs=2, space="PSUM") as ppool:
            for t in range(NT):
                cs = slice(t * P, (t + 1) * P)
                xa = pool.tile([P, P], F32)
                xb = pool.tile([P, P], F32)
                nc.sync.dma_start(out=xa[:], in_=xT_d[0:P, cs])
                nc.sync.dma_start(out=xb[:], in_=xT_d[P:2 * P, cs])
                ps = ppool.tile([P, HID], F32, space="PSUM")
                nc.tensor.matmul(out=ps[:], lhsT=xa[:], rhs=Wi_sb[:, 0:HID],
                                 start=True, stop=False)
                nc.tensor.matmul(out=ps[:], lhsT=xb[:], rhs=Wi_sb[:, HID:2 * HID],
                                 start=False, stop=False)
                nc.tensor.matmul(out=ps[:], lhsT=ones_sb, rhs=bi_sl,
                                 start=False, stop=True)
                nc.scalar.copy(out=h_sb[:, cs], in_=ps[:])

    # ================= layers =================
    for l in range(L):
        # ---- phase A: projections of local nodes ----
        with tile.TileContext(nc) as tc:
            with tc.tile_pool(name=f"A{l}", bufs=3) as pool, \
                 tc.tile_pool(name=f"Ap{l}", bufs=2, space="PSUM") as ppool:
                for t in range(NT):
                    cs = slice(t * P, (t + 1) * P)
                    pst = ppool.tile([P, P], F32, space="PSUM")
                    nc.tensor.transpose(out=pst[:], in_=h_sb[:, cs], identity=ident)
                    hT = pool.tile([P, P], F32)
                    nc.scalar.copy(out=hT[:], in_=pst[:])

                    pkv = ppool.tile([P, 2 * HID], F32, space="PSUM")
                    nc.tensor.matmul(out=pkv[:, 0:HID], lhsT=hT[:],
                                     rhs=Wslice(1, l), start=True, stop=False)
                    nc.tensor.matmul(out=pkv[:, 0:HID], lhsT=ones_sb,
                                     rhs=bslice(1, l), start=False, stop=True)
                    nc.tensor.matmul(out=pkv[:, HID:2 * HID], lhsT=hT[:],
                                     rhs=Wslice(2, l), start=True, stop=False)
                    nc.tensor.matmul(out=pkv[:, HID:2 * HID], lhsT=ones_sb,
                                     rhs=bslice(2, l), start=False, stop=True)
                    kvt = pool.tile([P, 2 * HID], F32)
                    nc.scalar.copy(out=kvt[:], in_=pkv[:])
                    nc.sync.dma_start(out=kv_in[t * P:(t + 1) * P, :], in_=kvt[:])

                    pqs = ppool.tile([P, 2 * HID], F32, space="PSUM")
                    nc.tensor.matmul(out=pqs[:, 0:HID], lhsT=hT[:],
                                     rhs=Wslice(0, l), start=True, stop=False)
                    nc.tensor.matmul(out=pqs[:, 0:HID], lhsT=ones_sb,
                                     rhs=bslice(0, l), start=False, stop=True)
                    nc.tensor.matmul(out=pqs[:, HID:2 * HID], lhsT=hT[:],
                                     rhs=Wslice(3, l), start=True, stop=False)
                    nc.tensor.matmul(out=pqs[:, HID:2 * HID], lhsT=ones_sb,
                                     rhs=bslice(3, l), start=False, stop=True)
                    nc.scalar.mul(out=q_sb[:, cs], in_=pqs[:, 0:HID], mul=RSC)
                    nc.scalar.copy(out=s_sb[:, cs], in_=pqs[:, HID:2 * HID])

        # ---- AllGather of KV (raw region) ----
        nc.gpsimd.collective_compute(
            "AllGather",
            mybir.AluOpType.bypass,
            ins=[kv_in[:]],
            outs=[kv_full[:]],
            replica_groups=[list(range(NCORES))],
        ).then_inc(cc_sem, 1)
        nc.gpsimd.wait_ge(cc_sem, l + 1)

        # ---- phase B: edge aggregation + node update ----
        with tile.TileContext(nc) as tc:
            with tc.tile_pool(name=f"B{l}", bufs=3) as pool, \
                 tc.tile_pool(name=f"Bw{l}", bufs=2) as wpool, \
                 tc.tile_pool(name=f"Bp{l}", bufs=2, space="PSUM") as ppool:
                for g in range(NT):
                    cs = slice(g * P, (g + 1) * P)
                    d = int(Dg[g])
                    o0 = int(offs[g])
                    cw = _chunks(d)
                    WL = min(d, JW)

                    # qWe[n,h] = sum_c q_s[n,hc]*We[hc]
                    qwe_s = wpool.tile([P, HID], F32)
                    nc.vector.tensor_tensor(out=qwe_s[:], in0=q_sb[:, cs],
                                            in1=repslice(0, l),
                                            op=mybir.AluOpType.mult)
                    qwe = wpool.tile([P, H], F32)
                    nc.vector.tensor_reduce(
                        out=qwe[:], in_=qwe_s[:].rearrange("p (h c) -> p h c", h=H),
                        axis=mybir.AxisListType.X, op=mybir.AluOpType.add)

                    num_w = wpool.tile([P, JW * HID], F32)
                    den_w = wpool.tile([P, JW * H], F32)
                    sden_w = wpool.tile([P, JW * H], F32)
                    if d == 0:
                        nc.vector.memset(num_w[:, 0:HID], 0.0)
                        nc.vector.memset(den_w[:, 0:H], 0.0)
                        nc.vector.memset(sden_w[:, 0:H], 0.0)

                    if d > 0:
                        idxg = pool.tile([P, d], I32)
                        nc.sync.dma_start(out=idxg[:], in_=gidx_d[:, o0:o0 + d])
                        eag = pool.tile([P, d], F32)
                        nc.sync.dma_start(out=eag[:], in_=eav_d[:, o0:o0 + d])
                        mkg = pool.tile([P, d], F32)
                        nc.sync.dma_start(out=mkg[:], in_=msk_d[:, o0:o0 + d])

                    joff = 0
                    for ci, w in enumerate(cw):
                        kvg = pool.tile([P, w * 2 * HID], F32)
                        for jj in range(w):
                            nc.gpsimd.indirect_dma_start(
                                out=kvg[:, jj * 2 * HID:(jj + 1) * 2 * HID],
                                out_offset=None,
                                in_=kv_full[:],
                                in_offset=bass.IndirectOffsetOnAxis(
                                    ap=idxg[:, joff + jj:joff + jj + 1], axis=0),
                            )
                        kj = kvg[:].rearrange("p (w f) -> p w f", w=w)[:, :, 0:HID]
                        vj = kvg[:].rearrange("p (w f) -> p w f", w=w)[:, :, HID:2 * HID]
                        qs = q_sb[:, cs]
                        qb = bass.AP(qs.tensor, qs.offset,
                                     [list(qs.ap[0]), [0, w], list(qs.ap[1])])
                        qk = pool.tile([P, w * HID], F32)
                        nc.vector.tensor_tensor(
                            out=qk[:].rearrange("p (w f) -> p w f", w=w),
                            in0=kj, in1=qb, op=mybir.AluOpType.mult)
                        alph = pool.tile([P, w * H], F32)
                        nc.vector.tensor_reduce(
                            out=alph[:],
                            in_=qk[:].rearrange("p (w h c) -> p w h c", w=w, h=H),
                            axis=mybir.AxisListType.X, op=mybir.AluOpType.add)
                        # + qWe*ea
                        eas = eag[:, joff:joff + w]
                        eab = bass.AP(eas.tensor, eas.offset,
                                      [list(eas.ap[0]), list(eas.ap[1]), [0, H]])
                        qweb = bass.AP(qwe[:].tensor, qwe[:].offset,
                                       [list(qwe[:].ap[0]), [0, w], list(qwe[:].ap[1])])
                        term = pool.tile([P, w * H], F32)
                        nc.vector.tensor_tensor(
                            out=term[:].rearrange("p (w h) -> p w h", w=w),
                            in0=eab, in1=qweb, op=mybir.AluOpType.mult)
                        nc.vector.tensor_tensor(out=alph[:], in0=alph[:],
                                                in1=term[:], op=mybir.AluOpType.add)
                        ex = pool.tile([P, w * H], F32)
                        nc.scalar.activation(out=ex[:], in_=alph[:],
                                             func=mybir.ActivationFunctionType.Exp)
                        mks = mkg[:, joff:joff + w]
                        mkb = bass.AP(mks.tensor, mks.offset,
                                      [list(mks.ap[0]), list(mks.ap[1]), [0, H]])
                        nc.vector.tensor_tensor(
                            out=ex[:].rearrange("p (w h) -> p w h", w=w),
                            in0=ex[:].rearrange("p (w h) -> p w h", w=w),
                            in1=mkb, op=mybir.AluOpType.mult)
                        # den / sden accumulation (lane-wise)
                        accop = mybir.AluOpType.add
                        if ci == 0:
                            nc.vector.tensor_copy(out=den_w[:, 0:w * H], in_=ex[:])
                        else:
                            nc.vector.tensor_tensor(out=den_w[:, 0:w * H],
                                                    in0=den_w[:, 0:w * H],
                                                    in1=ex[:], op=accop)
                        t2 = pool.tile([P, w * H], F32)
                        nc.vector.tensor_tensor(
                            out=t2[:].rearrange("p (w h) -> p w h", w=w),
                            in0=ex[:].rearrange("p (w h) -> p w h", w=w),
                            in1=eab, op=mybir.AluOpType.mult)
                        if ci == 0:
                            nc.vector.tensor_copy(out=sden_w[:, 0:w * H], in_=t2[:])
                        else:
                            nc.vector.tensor_tensor(out=sden_w[:, 0:w * H],
                                                    in0=sden_w[:, 0:w * H],
                                                    in1=t2[:], op=accop)
                        exb = bass.AP(ex[:].tensor, ex[:].offset,
                                      [list(ex[:].ap[0]), [H, w], [1, H], [0, C]])
                        exv = pool.tile([P, w * HID], F32)
                        nc.vector.tensor_tensor(
                            out=exv[:].rearrange("p (w f) -> p w f", w=w),
                            in0=vj, in1=exb, op=mybir.AluOpType.mult)
                        if ci == 0:
                            nc.vector.tensor_copy(out=num_w[:, 0:w * HID], in_=exv[:])
                        else:
                            nc.vector.tensor_tensor(out=num_w[:, 0:w * HID],
                                                    in0=num_w[:, 0:w * HID],
                                                    in1=exv[:], op=accop)
                        joff += w

                    # ---- finalize group ----
                    den = wpool.tile([P, H], F32)
                    if WL > 1:
                        nc.vector.tensor_reduce(
                            out=den[:],
                            in_=bass.AP(den_w[:].tensor, den_w[:].offset,
                                        [list(den_w[:].ap[0]), [1, H], [H, WL]]),
                            axis=mybir.AxisListType.X, op=mybir.AluOpType.add)
                    else:
                        nc.vector.tensor_copy(out=den[:], in_=den_w[:, 0:H])
                    nc.vector.tensor_scalar_add(den[:], den[:], 1e-16)
                    rden = wpool.tile([P, H], F32)
                    nc.vector.reciprocal(out=rden[:], in_=den[:])

                    sden = wpool.tile([P, H], F32)
                    if WL > 1:
                        nc.vector.tensor_reduce(
                            out=sden[:],
                            in_=bass.AP(sden_w[:].tensor, sden_w[:].offset,
                                        [list(sden_w[:].ap[0]), [1, H], [H, WL]]),
                            axis=mybir.AxisListType.X, op=mybir.AluOpType.add)
                    else:
                        nc.vector.tensor_copy(out=sden[:], in_=sden_w[:, 0:H])

                    num = wpool.tile([P, HID], F32)
                    if WL > 1:
                        nc.vector.tensor_reduce(
                            out=num[:],
                            in_=bass.AP(num_w[:].tensor, num_w[:].offset,
                                        [list(num_w[:].ap[0]), [1, HID], [HID, WL]]),
                            axis=mybir.AxisListType.X, op=mybir.AluOpType.add)
                    else:
                        nc.vector.tensor_copy(out=num[:], in_=num_w[:, 0:HID])
                    # num += sden_h * We_row
                    sdb = bass.AP(sden[:].tensor, sden[:].offset,
                                  [list(sden[:].ap[0]), [1, H], [0, C]])
                    fix = wpool.tile([P, HID], F32)
                    nc.vector.tensor_tensor(
                        out=fix[:].rearrange("p (h c) -> p h c", h=H),
                        in0=repslice(0, l).rearrange("p (h c) -> p h c", h=H),
                        in1=sdb, op=mybir.AluOpType.mult)
                    nc.vector.tensor_tensor(out=num[:], in0=num[:], in1=fix[:],
                                            op=mybir.AluOpType.add)
                    # out = num * rden_h
                    rdb = bass.AP(rden[:].tensor, rden[:].offset,
                                  [list(rden[:].ap[0]), [1, H], [0, C]])
                    outt = wpool.tile([P, HID], F32)
                    nc.vector.tensor_tensor(
                        out=outt[:].rearrange("p (h c) -> p h c", h=H),
                        in0=num[:].rearrange("p (h c) -> p h c", h=H),
                        in1=rdb, op=mybir.AluOpType.mult)

                    # gated skip: beta = sigmoid(out.wA + x_r.wB)
                    scr = wpool.tile([P, HID], F32)
                    dotA = wpool.tile([P, 1], F32)
                    nc.vector.tensor_tensor(out=scr[:], in0=outt[:],
                                            in1=repslice(1, l),
                                            op=mybir.AluOpType.mult)
                    nc.vector.tensor_reduce(out=dotA[:], in_=scr[:],
                                            axis=mybir.AxisListType.X,
                                            op=mybir.AluOpType.add)
                    scr2 = wpool.tile([P, HID], F32)
                    dotB = wpool.tile([P, 1], F32)
                    nc.vector.tensor_tensor(out=scr2[:], in0=s_sb[:, cs],
                                            in1=repslice(2, l),
                                            op=mybir.AluOpType.mult)
                    nc.vector.tensor_reduce(out=dotB[:], in_=scr2[:],
                                            axis=mybir.AxisListType.X,
                                            op=mybir.AluOpType.add)
                    blog = wpool.tile([P, 1], F32)
                    nc.vector.tensor_tensor(out=blog[:], in0=dotA[:], in1=dotB[:],
                                            op=mybir.AluOpType.add)
                    bet = wpool.tile([P, 1], F32)
                    nc.scalar.activation(out=bet[:], in_=blog[:],
                                         func=mybir.ActivationFunctionType.Sigmoid)
                    # conv = out + beta*(x_r - out)
                    d1 = wpool.tile([P, HID], F32)
                    nc.vector.tensor_tensor(out=d1[:], in0=s_sb[:, cs], in1=outt[:],
                                            op=mybir.AluOpType.subtract)
                    nc.vector.tensor_scalar(out=d1[:], in0=d1[:], scalar1=bet[:],
                                            scalar2=None, op0=mybir.AluOpType.mult)
                    conv = wpool.tile([P, HID], F32)
                    nc.vector.tensor_tensor(out=conv[:], in0=outt[:], in1=d1[:],
                                            op=mybir.AluOpType.add)
                    # gelu (exact / erf-based)
                    gl = wpool.tile([P, HID], F32)
                    nc.scalar.activation(out=gl[:], in_=conv[:],
                                         func=mybir.ActivationFunctionType.Gelu)
                    # residual + layernorm
                    x1 = wpool.tile([P, HID], F32)
                    nc.vector.tensor_tensor(out=x1[:], in0=gl[:], in1=h_sb[:, cs],
                                            op=mybir.AluOpType.add)
                    musum = wpool.tile([P, 1], F32)
                    nc.vector.tensor_reduce(out=musum[:], in_=x1[:],
                                            axis=mybir.AxisListType.X,
                                            op=mybir.AluOpType.add)
                    mu = wpool.tile([P, 1], F32)
                    nc.scalar.mul(out=mu[:], in_=musum[:], mul=1.0 / HID)
                    xc = wpool.tile([P, HID], F32)
                    nc.vector.tensor_scalar(out=xc[:], in0=x1[:], scalar1=mu[:],
                                            scalar2=None,
                                            op0=mybir.AluOpType.subtract)
                    sq = wpool.tile([P, HID], F32)
                    vsum = wpool.tile([P, 1], F32)
                    nc.scalar.activation(out=sq[:], in_=xc[:],
                                         func=mybir.ActivationFunctionType.Square,
                                         accum_out=vsum[:])
                    sd = wpool.tile([P, 1], F32)
                    nc.scalar.activation(out=sd[:], in_=vsum[:],
                                         func=mybir.ActivationFunctionType.Sqrt,
                                         scale=1.0 / HID, bias=eps_sb)
                    rstd = wpool.tile([P, 1], F32)
                    nc.vector.reciprocal(out=rstd[:], in_=sd[:])
                    xn = wpool.tile([P, HID], F32)
                    nc.vector.tensor_scalar(out=xn[:], in0=xc[:], scalar1=rstd[:],
                                            scalar2=None, op0=mybir.AluOpType.mult)
                    nc.vector.tensor_tensor(out=xn[:], in0=xn[:],
                                            in1=repslice(3, l),
                                            op=mybir.AluOpType.mult)
                    nc.vector.tensor_tensor(out=h_sb[:, cs], in0=xn[:],
                                            in1=repslice(4, l),
                                            op=mybir.AluOpType.add)
                    if l == L - 1:
                        ot = pool.tile([P, HID], F32)
                        nc.vector.tensor_copy(out=ot[:], in_=h_sb[:, cs])
                        nc.sync.dma_start(out=out_d[g * P:(g + 1) * P, :], in_=ot[:])

    nc.compile()
    _CACHE[key] = nc
    return nc


def kernel(x, edge_index, edge_attr, Wi, bi, Wq, bq, Wk, bk, Wv, bv, We,
           Wskip, bskip, Wbeta, ln_g, ln_b):
    x = np.asarray(x, np.float32)
    edge_index = np.asarray(edge_index, np.int32)
    edge_attr = np.asarray(edge_attr, np.float32)
    Wi = np.asarray(Wi, np.float32)
    bi = np.asarray(bi, np.float32)
    Wq = np.asarray(Wq, np.float32)
    bq = np.asarray(bq, np.float32)
    Wk = np.asarray(Wk, np.float32)
    bk = np.asarray(bk, np.float32)
    Wv = np.asarray(Wv, np.float32)
    bv = np.asarray(bv, np.float32)
    We = np.asarray(We, np.float32)
    Wskip = np.asarray(Wskip, np.float32)
    bskip = np.asarray(bskip, np.float32)
    Wbeta = np.asarray(Wbeta, np.float32)
    ln_g = np.asarray(ln_g, np.float32)
    ln_b = np.asarray(ln_b, np.float32)

    xT, gidx, eav, msk, Dg, offs, SUMD, perm = _preprocess(x, edge_index, edge_attr)

    WeV = We[:, 0, :]                                     # [L, HID]
    wA = Wbeta[:, 0:HID, 0] + Wbeta[:, 2 * HID:3 * HID, 0]
    wB = Wbeta[:, HID:2 * HID, 0] - Wbeta[:, 2 * HID:3 * HID, 0]

    def rep(a):  # [L, HID] -> [L, P, HID]
        return np.broadcast_to(a[:, None, :], (L, P, HID)).copy()

    common = {
        "Wi": Wi, "bi": bi[None, :],
        "Wq": Wq, "Wk": Wk, "Wv": Wv, "Ws": Wskip,
        "bq": bq[:, None, :], "bk": bk[:, None, :],
        "bv": bv[:, None, :], "bs": bskip[:, None, :],
        "WeR": rep(WeV), "wAR": rep(wA), "wBR": rep(wB),
        "lgR": rep(ln_g), "lbR": rep(ln_b),
    }
    in_maps = []
    for c in range(NCORES):
        m = dict(common)
        m["xT"] = xT[c]
        m["gidx"] = gidx[c]
        m["eav"] = eav[c]
        m["msk"] = msk[c]
        in_maps.append(m)

    nc = _build(Dg, offs, SUMD)
    global _last_in_maps
    _last_in_maps = in_maps
    res = bass_utils.run_bass_kernel_spmd(nc, in_maps,
                                          core_ids=list(range(NCORES)))
    out = np.empty((N, HID), np.float32)
    for c in range(NCORES):
        out[c * NLOC + perm[c]] = res.results[c]["out_h"][:NLOC]
    return out
